# revision 6
# baseline (speedup 1.0000x reference)
"""Bass/Tile kernel for AlphasFirstIRLSStep on 8 TRN2 NeuronCores.

Per-batch-problem pipeline (data-parallel over B=4096, 512 problems/core):
  W = 1/(|alpha|+eps)                              [N]
  Bk = G^T diag(W) G                               [9,9]   (TensorE, 14-problem packed)
  u  = min-eigenvector of Bk                       [9]     (sweep inverse + repeated squaring)
  out = |G @ u|                                    [N]     (DVE scalar_tensor_tensor chain)

Inputs are fed to the device in two layouts prepared host-side:
  gn [N, SB, 9]  n-major (feeds the Gram matmuls; contraction dim on partitions)
  gb [SB, N, 9]  b-major (feeds the output stage)
  an [N, SB]     n-major alpha
"""

import numpy as np

import concourse.bacc as bacc
import concourse.mybir as mybir
import concourse.tile as tile

F32 = mybir.dt.float32
ALU = mybir.AluOpType
ACTF = mybir.ActivationFunctionType
AX = mybir.AxisListType

IRLS_EPS = 1e-8
N_CORES = 8
B_FULL = 4096
N_FULL = 2000
D = 9

SB = B_FULL // N_CORES          # 512 problems per core
KC = 125                        # contraction chunk (partitions)
N_CHUNKS = N_FULL // KC         # 16
GROUPS = [14] * 36 + [8]        # problems per matmul group (9*14=126 cols)
GROUPS_PER_BLOCK = 7            # <= 8 PSUM banks
N_SQUARINGS = 13
OUT_CHUNK = 500                 # n-chunk for the output stage
N_BTILES = SB // 128            # 4 eigensolve tiles of 128 problems


def build_nc(sb=SB, n_full=N_FULL, kc=KC, groups=None, n_squarings=N_SQUARINGS,
             out_chunk=OUT_CHUNK):
    """Build the per-core Bass module (SPMD: same program on every core)."""
    if groups is None:
        groups = list(GROUPS)
    n_chunks = n_full // kc
    assert n_chunks * kc == n_full
    assert sum(groups) == sb
    n_btiles = (sb + 127) // 128

    # group -> (start problem, count); block -> list of group ids
    gstart = np.cumsum([0] + groups).tolist()
    blocks = [list(range(i, min(i + GROUPS_PER_BLOCK, len(groups))))
              for i in range(0, len(groups), GROUPS_PER_BLOCK)]

    nc = bacc.Bacc("TRN2", target_bir_lowering=False, debug=False,
                   num_devices=N_CORES)
    gn = nc.declare_dram_parameter("gn", [n_full, sb, D], F32, isOutput=False)
    gb = nc.declare_dram_parameter("gb", [sb, n_full, D], F32, isOutput=False)
    an = nc.declare_dram_parameter("an", [n_full, sb], F32, isOutput=False)
    out = nc.declare_dram_parameter("out", [sb, n_full], F32, isOutput=True)

    with tile.TileContext(nc) as tc:
        with (
            tc.tile_pool(name="wpool", bufs=n_chunks) as wpool,
            tc.tile_pool(name="wtmp", bufs=2) as wtmp,
            tc.tile_pool(name="gnp", bufs=3) as gnp,
            tc.tile_pool(name="wgp", bufs=3) as wgp,
            tc.tile_pool(name="psum", bufs=8, space="PSUM") as psum,
            tc.tile_pool(name="stage", bufs=3) as stagep,
            tc.tile_pool(name="eigB", bufs=n_btiles) as eigBp,
            tc.tile_pool(name="eigw", bufs=6) as eigw,
            tc.tile_pool(name="upool", bufs=n_btiles) as upool,
            tc.tile_pool(name="gbp", bufs=2) as gbp,
            tc.tile_pool(name="accp", bufs=4) as accp,
        ):
            # ---- W = 1/(|alpha|+eps), n-major [kc, sb] per chunk ----
            wts = []
            for c in range(n_chunks):
                a_t = wtmp.tile([kc, sb], F32, tag="a")
                nc.sync.dma_start(a_t[:], an[c * kc:(c + 1) * kc, :])
                ab_t = wtmp.tile([kc, sb], F32, tag="ab")
                nc.scalar.activation(ab_t[:], a_t[:], ACTF.Abs)
                ab2_t = wtmp.tile([kc, sb], F32, tag="ab2")
                nc.vector.tensor_scalar(out=ab2_t[:], in0=ab_t[:],
                                        scalar1=float(IRLS_EPS), scalar2=0.0,
                                        op0=ALU.add, op1=ALU.add)
                w_t = wpool.tile([kc, sb], F32)
                nc.vector.reciprocal(w_t[:], ab2_t[:])
                wts.append(w_t)

            # ---- Gram: per block of <=7 groups, accumulate over chunks ----
            eigB = [eigBp.tile([128, 81], F32, tag="eigB", name=f"eigB{i}")
                    for i in range(n_btiles)]
            for blk in blocks:
                b0 = gstart[blk[0]]
                bn = gstart[blk[-1] + 1] - b0     # problems in block
                cols = bn * D
                ps = [psum.tile([126, 126], F32, tag="gram", name=f"ps{g}")
                      for g in blk]
                for c in range(n_chunks):
                    g_t = gnp.tile([kc, cols], F32, tag="gn")
                    nc.sync.dma_start(g_t[:], gn[c * kc:(c + 1) * kc, b0:b0 + bn, :])
                    wg_t = wgp.tile([kc, cols], F32, tag="wg")
                    g3 = g_t[:].rearrange("p (b i) -> p b i", b=bn, i=D)
                    wg3 = wg_t[:].rearrange("p (b i) -> p b i", b=bn, i=D)
                    wb = wts[c][:, b0:b0 + bn].unsqueeze(2).broadcast_to([kc, bn, D])
                    nc.vector.tensor_tensor(out=wg3, in0=g3, in1=wb, op=ALU.mult)
                    for gi, g in enumerate(blk):
                        s0 = (gstart[g] - b0) * D
                        w = groups[g] * D
                        nc.tensor.matmul(
                            ps[gi][0:w, 0:w],
                            wg_t[:, s0:s0 + w],
                            g_t[:, s0:s0 + w],
                            start=(c == 0), stop=(c == n_chunks - 1),
                        )
                # extract diagonal 9x9 blocks -> eigB[b, 81]
                for gi, g in enumerate(blk):
                    w = groups[g] * D
                    st = stagep.tile([126, 126], F32, tag="stage")
                    nc.vector.tensor_copy(st[0:w, 0:w], ps[gi][0:w, 0:w])
                    for p in range(groups[g]):
                        b = gstart[g] + p
                        bt, br = divmod(b, 128)
                        nc.sync.dma_start(
                            eigB[bt][br:br + 1, :],
                            st[9 * p:9 * p + 9, 9 * p:9 * p + 9])

            # ---- eigensolve per 128-problem tile ----
            us = []
            for bt in range(n_btiles):
                rows = min(128, sb - bt * 128)
                A = eigB[bt]

                def A3(t, r=rows):
                    return t[0:r, :].rearrange("p (i j) -> p i j", i=D, j=D)

                # 9 sweeps: A <- sweep_p(A); after all: A == -inv(B)
                scol = eigw.tile([128, D], F32, tag="scol")
                rowp = eigw.tile([128, D], F32, tag="rowp")
                d = eigw.tile([128, 1], F32, tag="d")
                outer = eigw.tile([128, 81], F32, tag="outer")
                for p in range(D):
                    nc.vector.reciprocal(d[0:rows, :], A[0:rows, 10 * p:10 * p + 1])
                    nc.vector.tensor_scalar(
                        out=scol[0:rows, :], in0=A[0:rows, p::D],
                        scalar1=d[0:rows, :], scalar2=0.0,
                        op0=ALU.mult, op1=ALU.add)
                    nc.vector.tensor_scalar(
                        out=rowp[0:rows, :], in0=A[0:rows, D * p:D * p + D],
                        scalar1=d[0:rows, :], scalar2=0.0,
                        op0=ALU.mult, op1=ALU.add)
                    cb = scol[0:rows, :].unsqueeze(2).broadcast_to([rows, D, D])
                    rb = A[0:rows, D * p:D * p + D].unsqueeze(1).broadcast_to([rows, D, D])
                    nc.vector.tensor_tensor(out=A3(outer), in0=cb, in1=rb, op=ALU.mult)
                    nc.vector.tensor_tensor(out=A3(A), in0=A3(A), in1=A3(outer),
                                            op=ALU.subtract)
                    nc.vector.tensor_copy(A[0:rows, D * p:D * p + D], rowp[0:rows, :])
                    nc.vector.tensor_copy(A[0:rows, p::D], scol[0:rows, :])
                    nc.vector.tensor_scalar(
                        out=A[0:rows, 10 * p:10 * p + 1], in0=d[0:rows, :],
                        scalar1=-1.0, scalar2=0.0, op0=ALU.mult, op1=ALU.add)

                # trace-normalize, then repeated squaring
                tr = eigw.tile([128, 1], F32, tag="tr")
                rtr = eigw.tile([128, 1], F32, tag="rtr")
                M0 = eigw.tile([128, 81], F32, tag="m0")
                M1 = eigw.tile([128, 81], F32, tag="m1")
                tmp = eigw.tile([128, 81], F32, tag="mtmp")
                nc.vector.tensor_reduce(out=tr[0:rows, :], in_=A[0:rows, 0::10],
                                        axis=AX.X, op=ALU.add)
                nc.vector.reciprocal(rtr[0:rows, :], tr[0:rows, :])
                nc.vector.tensor_scalar(out=M0[0:rows, :], in0=A[0:rows, :],
                                        scalar1=rtr[0:rows, :], scalar2=0.0,
                                        op0=ALU.mult, op1=ALU.add)
                src, dst = M0, M1
                for _ in range(n_squarings):
                    for k in range(D):
                        ckb = src[0:rows, k::D].unsqueeze(2).broadcast_to([rows, D, D])
                        rkb = src[0:rows, D * k:D * k + D].unsqueeze(1).broadcast_to([rows, D, D])
                        if k == 0:
                            nc.vector.tensor_tensor(out=A3(dst), in0=ckb, in1=rkb,
                                                    op=ALU.mult)
                        else:
                            nc.vector.tensor_tensor(out=A3(tmp), in0=ckb, in1=rkb,
                                                    op=ALU.mult)
                            nc.vector.tensor_tensor(out=A3(dst), in0=A3(dst),
                                                    in1=A3(tmp), op=ALU.add)
                    nc.vector.tensor_reduce(out=tr[0:rows, :], in_=dst[0:rows, 0::10],
                                            axis=AX.X, op=ALU.add)
                    nc.vector.reciprocal(rtr[0:rows, :], tr[0:rows, :])
                    nc.vector.tensor_scalar(out=dst[0:rows, :], in0=dst[0:rows, :],
                                            scalar1=rtr[0:rows, :], scalar2=0.0,
                                            op0=ALU.mult, op1=ALU.add)
                    src, dst = dst, src

                # u = column of src with max diagonal, normalized
                dg = eigw.tile([128, D], F32, tag="dg")
                mx = eigw.tile([128, 1], F32, tag="mx")
                mask = eigw.tile([128, D], F32, tag="mask")
                u = upool.tile([128, D], F32)
                nc.vector.tensor_copy(dg[0:rows, :], src[0:rows, 0::10])
                nc.vector.tensor_reduce(out=mx[0:rows, :], in_=dg[0:rows, :],
                                        axis=AX.X, op=ALU.max)
                nc.vector.tensor_scalar(out=mask[0:rows, :], in0=dg[0:rows, :],
                                        scalar1=mx[0:rows, :], scalar2=0.0,
                                        op0=ALU.is_ge, op1=ALU.add)
                mb = mask[0:rows, :].unsqueeze(1).broadcast_to([rows, D, D])
                nc.vector.tensor_tensor(out=A3(tmp), in0=A3(src), in1=mb, op=ALU.mult)
                nc.vector.tensor_reduce(out=u[0:rows, :], in_=A3(tmp), axis=AX.X,
                                        op=ALU.add)
                usq = eigw.tile([128, D], F32, tag="usq")
                ss = eigw.tile([128, 1], F32, tag="ss")
                rs = eigw.tile([128, 1], F32, tag="rs")
                nc.vector.tensor_tensor(out=usq[0:rows, :], in0=u[0:rows, :],
                                        in1=u[0:rows, :], op=ALU.mult)
                nc.vector.tensor_reduce(out=ss[0:rows, :], in_=usq[0:rows, :],
                                        axis=AX.X, op=ALU.add)
                ssinv = eigw.tile([128, 1], F32, tag="ssinv")
                nc.vector.reciprocal(ssinv[0:rows, :], ss[0:rows, :])
                nc.scalar.activation(rs[0:rows, :], ssinv[0:rows, :], ACTF.Sqrt)
                nc.vector.tensor_scalar(out=u[0:rows, :], in0=u[0:rows, :],
                                        scalar1=rs[0:rows, :], scalar2=0.0,
                                        op0=ALU.mult, op1=ALU.add)
                us.append(u)

            # ---- output: |G @ u| in b-major layout ----
            n_ochunks = n_full // out_chunk
            assert n_ochunks * out_chunk == n_full
            for bt in range(n_btiles):
                rows = min(128, sb - bt * 128)
                u = us[bt]
                for oc in range(n_ochunks):
                    g_t = gbp.tile([128, out_chunk * D], F32, tag="gb")
                    nc.gpsimd.dma_start(
                        g_t[0:rows, :],
                        gb[bt * 128:bt * 128 + rows,
                           oc * out_chunk:(oc + 1) * out_chunk, :])
                    gv = g_t[0:rows, :].rearrange("p (n i) -> p n i",
                                                  n=out_chunk, i=D)
                    a1 = accp.tile([128, out_chunk], F32, tag="acc1")
                    a2 = accp.tile([128, out_chunk], F32, tag="acc2")
                    nc.vector.tensor_scalar(out=a1[0:rows, :], in0=gv[:, :, 0],
                                            scalar1=u[0:rows, 0:1], scalar2=0.0,
                                            op0=ALU.mult, op1=ALU.add)
                    s, t = a1, a2
                    for i in range(1, D):
                        nc.vector.scalar_tensor_tensor(
                            out=t[0:rows, :], in0=gv[:, :, i],
                            scalar=u[0:rows, i:i + 1], in1=s[0:rows, :],
                            op0=ALU.mult, op1=ALU.add)
                        s, t = t, s
                    fin = accp.tile([128, out_chunk], F32, tag="fin")
                    nc.scalar.activation(fin[0:rows, :], s[0:rows, :], ACTF.Abs)
                    nc.sync.dma_start(
                        out[bt * 128:bt * 128 + rows,
                            oc * out_chunk:(oc + 1) * out_chunk],
                        fin[0:rows, :])

    nc.compile()
    return nc


_NC_CACHE = {}


def _get_nc():
    if "nc" not in _NC_CACHE:
        _NC_CACHE["nc"] = build_nc()
    return _NC_CACHE["nc"]


def kernel(G, alpha_k, call_id=0):
    from concourse.bass_utils import run_bass_kernel_spmd

    G = np.ascontiguousarray(np.asarray(G, dtype=np.float32))
    alpha_k = np.asarray(alpha_k, dtype=np.float32)
    nc = _get_nc()

    in_maps = []
    for c in range(N_CORES):
        sl = slice(c * SB, (c + 1) * SB)
        gb_c = G[sl]                                            # [SB, N, 9]
        gn_c = np.ascontiguousarray(gb_c.transpose(1, 0, 2))    # [N, SB, 9]
        an_c = np.ascontiguousarray(alpha_k[sl].T)              # [N, SB]
        in_maps.append({"gn": gn_c, "gb": gb_c, "an": an_c})

    res = run_bass_kernel_spmd(nc, in_maps, list(range(N_CORES)))
    return np.concatenate([res.results[c]["out"] for c in range(N_CORES)], axis=0)


# revision 10
# speedup vs baseline: 1.5378x; 1.5378x over previous
"""Bass/Tile kernel for AlphasFirstIRLSStep on 8 TRN2 NeuronCores.

Per-batch-problem pipeline (data-parallel over B=4096, 512 problems/core):
  W = 1/(|alpha|+eps)                              [N]
  Bk = G^T diag(W) G                               [9,9]   (TensorE, 14-problem packed)
  u  = min-eigenvector of Bk                       [9]     (sweep inverse + squaring + 2D Ritz)
  out = |G @ u|                                    [N]     (DVE scalar_tensor_tensor chain)

Host-side layouts fed to the device:
  gn [N, SB, 9]  n-major G (Gram matmuls; contraction dim on partitions)
  gb [SB, 9, N]  i-major G (output stage; contiguous stt reads)
  an [N, SB]     n-major alpha
  dmask [128, 126] block-diagonal mask (zeroes cross-problem Gram garbage)

Gram results are extracted from the packed [126,126] PSUM by masking the
off-diagonal garbage, reducing over the 14-problem stride (-> [126, 9]
compact), bouncing through DRAM (contiguous per-problem 81-blocks), and
reading back [128, 81] eigensolve tiles.
"""

import numpy as np

import concourse.bacc as bacc
import concourse.mybir as mybir
import concourse.tile as tile

F32 = mybir.dt.float32
ALU = mybir.AluOpType
ACTF = mybir.ActivationFunctionType
AX = mybir.AxisListType

IRLS_EPS = 1e-8
N_CORES = 8
B_FULL = 4096
N_FULL = 2000
D = 9

SB = B_FULL // N_CORES          # 512 problems per core
KC = 125                        # contraction chunk (partitions)
GROUPS = [14] * 36 + [8]        # problems per matmul group (9*14=126 cols)
GROUPS_PER_BLOCK = 7            # <= 8 PSUM banks
N_SQUARINGS = 7                 # then 2D Rayleigh-Ritz refinement
OUT_CHUNK = 500                 # n-chunk for the output stage
RITZ_BIG = 1e30                 # (unused placeholder)


def build_nc(sb=SB, n_full=N_FULL, kc=KC, groups=None, n_squarings=N_SQUARINGS,
             out_chunk=OUT_CHUNK):
    if groups is None:
        groups = list(GROUPS)
    n_chunks = n_full // kc
    assert n_chunks * kc == n_full
    assert sum(groups) == sb
    n_btiles = (sb + 127) // 128
    n_groups = len(groups)

    gstart = np.cumsum([0] + groups).tolist()
    blocks = [list(range(i, min(i + GROUPS_PER_BLOCK, n_groups)))
              for i in range(0, n_groups, GROUPS_PER_BLOCK)]

    nc = bacc.Bacc("TRN2", target_bir_lowering=False, debug=False,
                   num_devices=N_CORES)
    gn = nc.declare_dram_parameter("gn", [n_full, sb, D], F32, isOutput=False)
    gb = nc.declare_dram_parameter("gb", [sb, D, n_full], F32, isOutput=False)
    an = nc.declare_dram_parameter("an", [n_full, sb], F32, isOutput=False)
    dmask = nc.declare_dram_parameter("dmask", [128, 126], F32, isOutput=False)
    out = nc.declare_dram_parameter("out", [sb, n_full], F32, isOutput=True)

    with tile.TileContext(nc) as tc:
        with (
            tc.tile_pool(name="wpool", bufs=n_chunks) as wpool,
            tc.tile_pool(name="wtmp", bufs=3) as wtmp,
            tc.tile_pool(name="gnp", bufs=3) as gnp,
            tc.tile_pool(name="wgp", bufs=3) as wgp,
            tc.tile_pool(name="psum", bufs=8, space="PSUM") as psum,
            tc.tile_pool(name="extr", bufs=4) as extr,
            tc.tile_pool(name="drp", bufs=1, space="DRAM") as drp,
            tc.tile_pool(name="eigBp", bufs=n_btiles) as eigBp,
            tc.tile_pool(name="eigw", bufs=8) as eigw,
            tc.tile_pool(name="upool", bufs=n_btiles) as upool,
            tc.tile_pool(name="gbp", bufs=3) as gbp,
            tc.tile_pool(name="accp", bufs=6) as accp,
        ):
            mk = wtmp.tile([128, 126], F32, tag="dmask")
            nc.sync.dma_start(mk[:], dmask[:])

            # ---- W = 1/(|alpha|+eps), n-major [kc, sb] per chunk ----
            wts = []
            for c in range(n_chunks):
                a_t = wtmp.tile([kc, sb], F32, tag="a")
                nc.sync.dma_start(a_t[:], an[c * kc:(c + 1) * kc, :])
                ab_t = wtmp.tile([kc, sb], F32, tag="ab")
                nc.scalar.activation(ab_t[:], a_t[:], ACTF.Abs)
                ab2_t = wtmp.tile([kc, sb], F32, tag="ab2")
                nc.vector.tensor_scalar(out=ab2_t[:], in0=ab_t[:],
                                        scalar1=float(IRLS_EPS), scalar2=0.0,
                                        op0=ALU.add, op1=ALU.add)
                w_t = wpool.tile([kc, sb], F32)
                nc.vector.reciprocal(w_t[:], ab2_t[:])
                wts.append(w_t)

            # ---- Gram + extraction ----
            # per-group DRAM bounce tiles so each readback only depends on
            # its own group's write (keeps Gram -> eig pipelined)
            bstage = [drp.tile([126, D], F32, tag=f"bst{g}", name=f"bst{g}")
                      for g in range(n_groups)]
            for blk in blocks:
                b0 = gstart[blk[0]]
                bn = gstart[blk[-1] + 1] - b0
                cols = bn * D
                ps = [psum.tile([126, 126], F32, tag="gram", name=f"ps{g}")
                      for g in blk]
                for c in range(n_chunks):
                    g_t = gnp.tile([kc, cols], F32, tag="gn")
                    nc.sync.dma_start(g_t[:], gn[c * kc:(c + 1) * kc, b0:b0 + bn, :])
                    wg_t = wgp.tile([kc, cols], F32, tag="wg")
                    g3 = g_t[:].rearrange("p (b i) -> p b i", b=bn, i=D)
                    wg3 = wg_t[:].rearrange("p (b i) -> p b i", b=bn, i=D)
                    wb = wts[c][:, b0:b0 + bn].unsqueeze(2).broadcast_to([kc, bn, D])
                    nc.vector.tensor_tensor(out=wg3, in0=g3, in1=wb, op=ALU.mult)
                    for gi, g in enumerate(blk):
                        s0 = (gstart[g] - b0) * D
                        w = groups[g] * D
                        nc.tensor.matmul(
                            ps[gi][0:w, 0:w],
                            wg_t[:, s0:s0 + w],
                            g_t[:, s0:s0 + w],
                            start=(c == 0), stop=(c == n_chunks - 1),
                        )
                # mask garbage, reduce over problem-stride, bounce to DRAM
                for gi, g in enumerate(blk):
                    ng = groups[g]
                    w = ng * D
                    sm = extr.tile([126, 126], F32, tag="sm")
                    nc.vector.tensor_tensor(out=sm[0:w, 0:w], in0=ps[gi][0:w, 0:w],
                                            in1=mk[0:w, 0:w], op=ALU.mult)
                    cm = extr.tile([126, D], F32, tag="cm")
                    smv = sm[0:w, 0:w].rearrange("q (p j) -> q j p", p=ng, j=D)
                    nc.vector.tensor_reduce(out=cm[0:w, :], in_=smv, axis=AX.X,
                                            op=ALU.add)
                    nc.sync.dma_start(bstage[g][0:w, :], cm[0:w, :])

            # readback: contiguous per-problem 81-blocks -> [128, 81] tiles
            eigB = [eigBp.tile([128, 81], F32, tag="eigB", name=f"eigB{i}")
                    for i in range(n_btiles)]
            for bt in range(n_btiles):
                lo_t, hi_t = bt * 128, min(bt * 128 + 128, sb)
                for g in range(n_groups):
                    gs, ge = gstart[g], gstart[g + 1]
                    lo, hi = max(gs, lo_t), min(ge, hi_t)
                    if lo >= hi:
                        continue
                    cnt = hi - lo
                    src = bstage[g][(lo - gs) * D:(lo - gs + cnt) * D, :]
                    nc.sync.dma_start(
                        eigB[bt][lo - lo_t:lo - lo_t + cnt, :],
                        src.rearrange("(b r) j -> b (r j)", r=D))

            # ---- eigensolve per 128-problem tile ----
            us = []
            for bt in range(n_btiles):
                rows = min(128, sb - bt * 128)
                Bt = eigB[bt]

                def A3(t, r=rows):
                    return t[0:r, :].rearrange("p (i j) -> p i j", i=D, j=D)

                def matvec(dst, Msrc, vsrc, r=rows, tmp_tag="mvtmp"):
                    # dst[b, i] = sum_j Msrc[b, i, j] * vsrc[b, j]
                    t = eigw.tile([128, 81], F32, tag=tmp_tag)
                    vb = vsrc[0:r, :].unsqueeze(1).broadcast_to([r, D, D])
                    nc.vector.tensor_tensor(out=A3(t, r), in0=A3(Msrc, r), in1=vb,
                                            op=ALU.mult)
                    nc.vector.tensor_reduce(out=dst[0:r, :], in_=A3(t, r),
                                            axis=AX.X, op=ALU.add)

                def dot(dst, x, y, r=rows, tmp_tag="dottmp"):
                    t = eigw.tile([128, D], F32, tag=tmp_tag)
                    nc.vector.tensor_tensor(out=t[0:r, :], in0=x[0:r, :],
                                            in1=y[0:r, :], op=ALU.mult)
                    nc.vector.tensor_reduce(out=dst[0:r, :], in_=t[0:r, :],
                                            axis=AX.X, op=ALU.add)

                # sweeps operate on a copy; Bt stays intact for the Ritz step
                A = eigw.tile([128, 81], F32, tag="Awork")
                nc.vector.tensor_copy(A[0:rows, :], Bt[0:rows, :])
                scol = eigw.tile([128, D], F32, tag="scol")
                rowp = eigw.tile([128, D], F32, tag="rowp")
                d = eigw.tile([128, 1], F32, tag="d")
                outer = eigw.tile([128, 81], F32, tag="outer")
                for p in range(D):
                    nc.vector.reciprocal(d[0:rows, :], A[0:rows, 10 * p:10 * p + 1])
                    nc.vector.tensor_scalar(
                        out=scol[0:rows, :], in0=A[0:rows, p::D],
                        scalar1=d[0:rows, :], scalar2=0.0,
                        op0=ALU.mult, op1=ALU.add)
                    nc.vector.tensor_scalar(
                        out=rowp[0:rows, :], in0=A[0:rows, D * p:D * p + D],
                        scalar1=d[0:rows, :], scalar2=0.0,
                        op0=ALU.mult, op1=ALU.add)
                    cb = scol[0:rows, :].unsqueeze(2).broadcast_to([rows, D, D])
                    rb = A[0:rows, D * p:D * p + D].unsqueeze(1).broadcast_to([rows, D, D])
                    nc.vector.tensor_tensor(out=A3(outer), in0=cb, in1=rb, op=ALU.mult)
                    nc.vector.tensor_tensor(out=A3(A), in0=A3(A), in1=A3(outer),
                                            op=ALU.subtract)
                    nc.vector.tensor_copy(A[0:rows, D * p:D * p + D], rowp[0:rows, :])
                    nc.vector.tensor_copy(A[0:rows, p::D], scol[0:rows, :])
                    nc.vector.tensor_scalar(
                        out=A[0:rows, 10 * p:10 * p + 1], in0=d[0:rows, :],
                        scalar1=-1.0, scalar2=0.0, op0=ALU.mult, op1=ALU.add)

                # trace-normalize + repeated squaring (rescale every 2nd)
                tr = eigw.tile([128, 1], F32, tag="tr")
                rtr = eigw.tile([128, 1], F32, tag="rtr")
                M0 = eigw.tile([128, 81], F32, tag="m0")
                M1 = eigw.tile([128, 81], F32, tag="m1")
                tmp = eigw.tile([128, 81], F32, tag="mtmp")
                nc.vector.tensor_reduce(out=tr[0:rows, :], in_=A[0:rows, 0::10],
                                        axis=AX.X, op=ALU.add)
                nc.vector.reciprocal(rtr[0:rows, :], tr[0:rows, :])
                nc.vector.tensor_scalar(out=M0[0:rows, :], in0=A[0:rows, :],
                                        scalar1=rtr[0:rows, :], scalar2=0.0,
                                        op0=ALU.mult, op1=ALU.add)
                src, dst = M0, M1
                for k in range(n_squarings):
                    for kk in range(D):
                        ckb = src[0:rows, kk::D].unsqueeze(2).broadcast_to([rows, D, D])
                        rkb = src[0:rows, D * kk:D * kk + D].unsqueeze(1).broadcast_to([rows, D, D])
                        if kk == 0:
                            nc.vector.tensor_tensor(out=A3(dst), in0=ckb, in1=rkb,
                                                    op=ALU.mult)
                        else:
                            nc.vector.tensor_tensor(out=A3(tmp), in0=ckb, in1=rkb,
                                                    op=ALU.mult)
                            nc.vector.tensor_tensor(out=A3(dst), in0=A3(dst),
                                                    in1=A3(tmp), op=ALU.add)
                    if (k % 2 == 1) or k == n_squarings - 1:
                        nc.vector.tensor_reduce(out=tr[0:rows, :],
                                                in_=dst[0:rows, 0::10],
                                                axis=AX.X, op=ALU.add)
                        nc.vector.reciprocal(rtr[0:rows, :], tr[0:rows, :])
                        nc.vector.tensor_scalar(out=dst[0:rows, :], in0=dst[0:rows, :],
                                                scalar1=rtr[0:rows, :], scalar2=0.0,
                                                op0=ALU.mult, op1=ALU.add)
                    src, dst = dst, src

                # ---- 2D Rayleigh-Ritz refinement on span{v1, v2} ----
                Mp = src
                dg = eigw.tile([128, D], F32, tag="dg")
                m1_ = eigw.tile([128, 1], F32, tag="m1_")
                mask1 = eigw.tile([128, D], F32, tag="mask1")
                dg2 = eigw.tile([128, D], F32, tag="dg2")
                m2_ = eigw.tile([128, 1], F32, tag="m2_")
                mask2 = eigw.tile([128, D], F32, tag="mask2")
                nc.vector.tensor_copy(dg[0:rows, :], Mp[0:rows, 0::10])
                nc.vector.tensor_reduce(out=m1_[0:rows, :], in_=dg[0:rows, :],
                                        axis=AX.X, op=ALU.max)
                nc.vector.tensor_scalar(out=mask1[0:rows, :], in0=dg[0:rows, :],
                                        scalar1=m1_[0:rows, :], scalar2=0.0,
                                        op0=ALU.is_ge, op1=ALU.add)
                # dg2 = dg - mask1*2 (diag of trace-normalized PSD is in [0,1])
                nc.vector.scalar_tensor_tensor(out=dg2[0:rows, :], in0=mask1[0:rows, :],
                                               scalar=-2.0, in1=dg[0:rows, :],
                                               op0=ALU.mult, op1=ALU.add)
                nc.vector.tensor_reduce(out=m2_[0:rows, :], in_=dg2[0:rows, :],
                                        axis=AX.X, op=ALU.max)
                nc.vector.tensor_scalar(out=mask2[0:rows, :], in0=dg2[0:rows, :],
                                        scalar1=m2_[0:rows, :], scalar2=0.0,
                                        op0=ALU.is_ge, op1=ALU.add)
                v1 = eigw.tile([128, D], F32, tag="v1")
                v2 = eigw.tile([128, D], F32, tag="v2")
                matvec(v1, Mp, mask1)
                matvec(v2, Mp, mask2)
                # n1 = v1/||v1||
                ss = eigw.tile([128, 1], F32, tag="ss")
                rs = eigw.tile([128, 1], F32, tag="rs")
                rsq = eigw.tile([128, 1], F32, tag="rsq")
                n1 = eigw.tile([128, D], F32, tag="n1")
                dot(ss, v1, v1)
                nc.vector.reciprocal(rs[0:rows, :], ss[0:rows, :])
                nc.scalar.activation(rsq[0:rows, :], rs[0:rows, :], ACTF.Sqrt)
                nc.vector.tensor_scalar(out=n1[0:rows, :], in0=v1[0:rows, :],
                                        scalar1=rsq[0:rows, :], scalar2=0.0,
                                        op0=ALU.mult, op1=ALU.add)
                # w = (n1*c - v2), re-orthogonalized twice (sign flips are fine)
                cc = eigw.tile([128, 1], F32, tag="cc")
                w_ = eigw.tile([128, D], F32, tag="w_")
                dot(cc, n1, v2)
                nc.vector.scalar_tensor_tensor(out=w_[0:rows, :], in0=n1[0:rows, :],
                                               scalar=cc[0:rows, :], in1=v2[0:rows, :],
                                               op0=ALU.mult, op1=ALU.subtract)
                dot(cc, n1, w_)
                nc.vector.scalar_tensor_tensor(out=w_[0:rows, :], in0=n1[0:rows, :],
                                               scalar=cc[0:rows, :], in1=w_[0:rows, :],
                                               op0=ALU.mult, op1=ALU.subtract)
                # n2 = w/sqrt(||w||^2 + tiny)
                ssw = eigw.tile([128, 1], F32, tag="ssw")
                n2 = eigw.tile([128, D], F32, tag="n2")
                dot(ssw, w_, w_)
                nc.vector.tensor_scalar(out=ssw[0:rows, :], in0=ssw[0:rows, :],
                                        scalar1=1e-30, scalar2=0.0,
                                        op0=ALU.add, op1=ALU.add)
                nc.vector.reciprocal(rs[0:rows, :], ssw[0:rows, :])
                nc.scalar.activation(rsq[0:rows, :], rs[0:rows, :], ACTF.Sqrt)
                nc.vector.tensor_scalar(out=n2[0:rows, :], in0=w_[0:rows, :],
                                        scalar1=rsq[0:rows, :], scalar2=0.0,
                                        op0=ALU.mult, op1=ALU.add)
                ssn2 = eigw.tile([128, 1], F32, tag="ssn2")
                dot(ssn2, n2, n2)
                # Rayleigh 2x2 on original B
                Bn1 = eigw.tile([128, D], F32, tag="Bn1")
                Bn2 = eigw.tile([128, D], F32, tag="Bn2")
                aq = eigw.tile([128, 1], F32, tag="aq")
                bq = eigw.tile([128, 1], F32, tag="bq")
                cq = eigw.tile([128, 1], F32, tag="cq")
                matvec(Bn1, Bt, n1)
                matvec(Bn2, Bt, n2)
                dot(aq, n1, Bn1)
                dot(bq, n1, Bn2)
                dot(cq, n2, Bn2)
                # cq += (1 - ssn2) * (aq + cq + 1)   [aq, cq >= 0 for SPD]
                pen = eigw.tile([128, 1], F32, tag="pen")
                mag = eigw.tile([128, 1], F32, tag="mag")
                nc.vector.tensor_scalar(out=pen[0:rows, :], in0=ssn2[0:rows, :],
                                        scalar1=-1.0, scalar2=1.0,
                                        op0=ALU.mult, op1=ALU.add)
                nc.vector.tensor_tensor(out=mag[0:rows, :], in0=aq[0:rows, :],
                                        in1=cq[0:rows, :], op=ALU.add)
                nc.vector.tensor_scalar(out=mag[0:rows, :], in0=mag[0:rows, :],
                                        scalar1=1.0, scalar2=0.0,
                                        op0=ALU.add, op1=ALU.add)
                nc.vector.tensor_tensor(out=pen[0:rows, :], in0=pen[0:rows, :],
                                        in1=mag[0:rows, :], op=ALU.mult)
                nc.vector.tensor_tensor(out=cq[0:rows, :], in0=cq[0:rows, :],
                                        in1=pen[0:rows, :], op=ALU.add)
                # lam = (aq+cq)/2 - sqrt(((aq-cq)/2)^2 + bq^2)
                h = eigw.tile([128, 1], F32, tag="h")
                r2 = eigw.tile([128, 1], F32, tag="r2")
                lam = eigw.tile([128, 1], F32, tag="lam")
                nc.vector.tensor_tensor(out=h[0:rows, :], in0=aq[0:rows, :],
                                        in1=cq[0:rows, :], op=ALU.subtract)
                nc.vector.tensor_scalar(out=h[0:rows, :], in0=h[0:rows, :],
                                        scalar1=0.5, scalar2=0.0,
                                        op0=ALU.mult, op1=ALU.add)
                nc.vector.tensor_tensor(out=r2[0:rows, :], in0=h[0:rows, :],
                                        in1=h[0:rows, :], op=ALU.mult)
                nc.vector.scalar_tensor_tensor(out=r2[0:rows, :], in0=bq[0:rows, :],
                                               scalar=bq[0:rows, :], in1=r2[0:rows, :],
                                               op0=ALU.mult, op1=ALU.add)
                nc.scalar.activation(r2[0:rows, :], r2[0:rows, :], ACTF.Sqrt)
                nc.vector.tensor_tensor(out=lam[0:rows, :], in0=aq[0:rows, :],
                                        in1=cq[0:rows, :], op=ALU.add)
                nc.vector.scalar_tensor_tensor(out=lam[0:rows, :], in0=lam[0:rows, :],
                                               scalar=0.5, in1=r2[0:rows, :],
                                               op0=ALU.mult, op1=ALU.subtract)
                # candidates: v1c=(bq, lam-aq), v2c=(lam-cq, bq); pick v2c iff aq<=cq
                x1 = bq
                y1 = eigw.tile([128, 1], F32, tag="y1")
                x2 = eigw.tile([128, 1], F32, tag="x2")
                pm = eigw.tile([128, 1], F32, tag="pm")
                nc.vector.tensor_tensor(out=y1[0:rows, :], in0=lam[0:rows, :],
                                        in1=aq[0:rows, :], op=ALU.subtract)
                nc.vector.tensor_tensor(out=x2[0:rows, :], in0=lam[0:rows, :],
                                        in1=cq[0:rows, :], op=ALU.subtract)
                nc.vector.tensor_tensor(out=pm[0:rows, :], in0=aq[0:rows, :],
                                        in1=cq[0:rows, :], op=ALU.is_le)
                # x = x1 + pm*(x2-x1); y = y1 + pm*(y2-y1) with y2 = bq = x1
                xf = eigw.tile([128, 1], F32, tag="xf")
                yf = eigw.tile([128, 1], F32, tag="yf")
                dx = eigw.tile([128, 1], F32, tag="dx")
                nc.vector.tensor_tensor(out=dx[0:rows, :], in0=x2[0:rows, :],
                                        in1=x1[0:rows, :], op=ALU.subtract)
                nc.vector.scalar_tensor_tensor(out=xf[0:rows, :], in0=dx[0:rows, :],
                                               scalar=pm[0:rows, :], in1=x1[0:rows, :],
                                               op0=ALU.mult, op1=ALU.add)
                nc.vector.tensor_tensor(out=dx[0:rows, :], in0=x1[0:rows, :],
                                        in1=y1[0:rows, :], op=ALU.subtract)
                nc.vector.scalar_tensor_tensor(out=yf[0:rows, :], in0=dx[0:rows, :],
                                               scalar=pm[0:rows, :], in1=y1[0:rows, :],
                                               op0=ALU.mult, op1=ALU.add)
                # u = xf*n1 + yf*n2, normalized
                u = upool.tile([128, D], F32)
                ut = eigw.tile([128, D], F32, tag="ut")
                nc.vector.tensor_scalar(out=ut[0:rows, :], in0=n1[0:rows, :],
                                        scalar1=xf[0:rows, :], scalar2=0.0,
                                        op0=ALU.mult, op1=ALU.add)
                nc.vector.scalar_tensor_tensor(out=u[0:rows, :], in0=n2[0:rows, :],
                                               scalar=yf[0:rows, :], in1=ut[0:rows, :],
                                               op0=ALU.mult, op1=ALU.add)
                dot(ss, u, u)
                nc.vector.reciprocal(rs[0:rows, :], ss[0:rows, :])
                nc.scalar.activation(rsq[0:rows, :], rs[0:rows, :], ACTF.Sqrt)
                nc.vector.tensor_scalar(out=u[0:rows, :], in0=u[0:rows, :],
                                        scalar1=rsq[0:rows, :], scalar2=0.0,
                                        op0=ALU.mult, op1=ALU.add)
                us.append(u)

            # ---- output: |G @ u|, i-major gb so stt reads are contiguous ----
            n_ochunks = n_full // out_chunk
            assert n_ochunks * out_chunk == n_full
            for bt in range(n_btiles):
                rows = min(128, sb - bt * 128)
                u = us[bt]
                for oc in range(n_ochunks):
                    g_t = gbp.tile([128, D * out_chunk], F32, tag="gb")
                    nc.gpsimd.dma_start(
                        g_t[0:rows, :],
                        gb[bt * 128:bt * 128 + rows, :,
                           oc * out_chunk:(oc + 1) * out_chunk])
                    a1 = accp.tile([128, out_chunk], F32, tag="acc1")
                    a2 = accp.tile([128, out_chunk], F32, tag="acc2")
                    nc.vector.tensor_scalar(
                        out=a1[0:rows, :], in0=g_t[0:rows, 0:out_chunk],
                        scalar1=u[0:rows, 0:1], scalar2=0.0,
                        op0=ALU.mult, op1=ALU.add)
                    s, t = a1, a2
                    for i in range(1, D):
                        nc.vector.scalar_tensor_tensor(
                            out=t[0:rows, :],
                            in0=g_t[0:rows, i * out_chunk:(i + 1) * out_chunk],
                            scalar=u[0:rows, i:i + 1], in1=s[0:rows, :],
                            op0=ALU.mult, op1=ALU.add)
                        s, t = t, s
                    fin = accp.tile([128, out_chunk], F32, tag="fin")
                    nc.scalar.activation(fin[0:rows, :], s[0:rows, :], ACTF.Abs)
                    nc.sync.dma_start(
                        out[bt * 128:bt * 128 + rows,
                            oc * out_chunk:(oc + 1) * out_chunk],
                        fin[0:rows, :])

    nc.compile()
    return nc


def make_dmask():
    mk = np.zeros((128, 126), dtype=np.float32)
    for p in range(14):
        mk[9 * p:9 * p + 9, 9 * p:9 * p + 9] = 1.0
    return mk


_NC_CACHE = {}


def _get_nc():
    if "nc" not in _NC_CACHE:
        _NC_CACHE["nc"] = build_nc()
    return _NC_CACHE["nc"]


def kernel(G, alpha_k, call_id=0):
    from concourse.bass_utils import run_bass_kernel_spmd

    G = np.ascontiguousarray(np.asarray(G, dtype=np.float32))
    alpha_k = np.asarray(alpha_k, dtype=np.float32)
    nc = _get_nc()
    mk = make_dmask()

    in_maps = []
    for c in range(N_CORES):
        sl = slice(c * SB, (c + 1) * SB)
        gs = G[sl]
        gn_c = np.ascontiguousarray(gs.transpose(1, 0, 2))    # [N, SB, 9]
        gb_c = np.ascontiguousarray(gs.transpose(0, 2, 1))    # [SB, 9, N]
        an_c = np.ascontiguousarray(alpha_k[sl].T)            # [N, SB]
        in_maps.append({"gn": gn_c, "gb": gb_c, "an": an_c, "dmask": mk})

    res = run_bass_kernel_spmd(nc, in_maps, list(range(N_CORES)))
    return np.concatenate([res.results[c]["out"] for c in range(N_CORES)], axis=0)
